# revision 29
# baseline (speedup 1.0000x reference)
"""3x3 zero-padded window NMS (CenterNet points) on 8 trn2 NeuronCores.

points: [16, 80, 128, 128] f32 in [0,1).  out = where(p == 3x3_local_max, p, 0).

Strategy
--------
Pure data parallel over the 1280 (b,c) planes: core k owns planes
[160k, 160k+160).  Host zero-pads each plane to 130x130 so the kernel has
no edge cases.

Per-core layout: planes on SBUF partitions.  A tile covers 32 planes x
4 vertical strips (= 128 partitions), each strip 32 output rows + 2 halo
rows, full 130-col width.  All shifts are free-dim AP shifts.

Compute (per tile, all exact fp32, all on DVE).  The DVE is the only
engine with 2-tensor elementwise ops (ACT bias/scale are per-partition
scalars; GPSIMD TensorTensor is rejected by walrus codegen on Pool), and
fp32 tensor_tensor runs at 1 elem/cycle/lane, so the cycle count is the
total number of output elements across sweeps.  A pair-max decomposition
of the 3-tap sliding max cuts that from 2/elem to 1.5/elem per direction:

  vertical   Q[k]     = max(t[2k+1], t[2k+2])            k = 0..15
             V3[2k]   = max(t[2k],   Q[k])       (rows 2k..2k+2)
             V3[2k+1] = max(Q[k],    t[2k+3])    (rows 2k+1..2k+3)
  horizontal Ph[m]    = max(V3[:,2m+1], V3[:,2m+2])      m = 0..63
             V[2m]    = max(V3[:,2m], Ph[m])     (cols 2m..2m+2)
             V[2m+1]  = max(Ph[m], V3[:,2m+3])   (cols 2m+1..2m+3)
  out = select(V - p < 2^-24, p, 0)              fused custom DVE op

16480 cycles/group vs 20770 for the plain separable 2+2-pass form.  Every
sweep writes a tile it does not read: an in-place V3 update was measured
~16% SLOWER end-to-end (read+write streams on the same SBUF bank).  Ph is
aliased into Qv's storage (2080 elems/partition >= 2048; Q is dead once
V3 is built) to keep the footprint under the SBUF budget.  (A chained-
select variant -- out = SEL(V3_interleaved, SEL(Ph_expanded, p)) -- would
drop one instruction at equal FD, but _custom_dve APs are capped at 2 free
dims and the expanded/interleaved reads need 3.)
Inputs are multiples of 2^-23 (jax.random.uniform), so V - p is exact in
fp32: 0 iff p is the window max, else >= 2^-23 -> the select is bit-exact.

Perf notes (HW-measured):
 - The DVE stalls ~op-duration when an op consumes the *immediately*
   previous op's output; distance >= 2 streams at full rate.  Two groups
   are processed interleaved (Qg Qh Eg Og Eh Oh Pg Ph Veg Veh Vog Voh Sg
   Sh) so every op is full-size, 7 instructions/group, and every
   producer->consumer pair is >= 2 instructions apart -- halving the
   ~151-cycle-per-instruction init overhead vs split-half staggering.
   An odd trailing group falls back to the 14-half-instruction order.
 - DMA APs keep the 32-plane dim outermost (HWDGE ring fan-out keys on it;
   3x bandwidth vs strip-outermost).
 - Loads prefetch 2 groups ahead and are emitted before stores so the
   in-order SP queue never holds a needed load behind a store's wait.

Why this is the roofline (HW-bisected; all numbers per-iteration slope):
 - mode=pure (compute only, no DMA): 83.4us -> real DVE ~1.0 GHz, 82400
   cycles/core of work + ~1us init.  The DVE has 2x4B SBUF read ports;
   every fp32 2-src op is port-bound at 1 out/cycle.  The sweep structure
   reads 8.05 B/port-pair per output elem = the 2-src-op lower bound
   (vertical 1.5 + horizontal 1.5 + select 1.0 passes/elem).
 - Faster modes exist only for 1-src ops (InstTensorScalarPtr/TensorCopy
   support 2x_2p fp32 = 2 elem/cycle) -- nothing in NMS is 1-src.
   TensorTensor supports only 2x_1p (needs 2-byte dtype); 16-bit maxes
   break exactness (measured-style est. rel err ~0.2 vs 2e-2 gate: a
   false peak needs ~18 mantissa bits to stay under tolerance).
   InstTensorReduce/InstPool read 1 elem/cycle (no modes) -> worse.
   Custom DVE ops report no perf modes regardless of shape/subdim.
 - GPSIMD: walrus rejects ALL 2-src ops on Pool (TensorTensor AND
   scalar_tensor_tensor); only tensor_scalar/copy pass -> no offload.
   ACT: bias/scale per-partition scalars only -> no offload.
 - mode=dmaonly: 69.2us (21.8 MB @ ~315 GB/s) -> DMA fully hidden.
 - full - pure = 4-12us: DMA<->DVE SBUF bank contention, sensitive to
   burst pattern.  Variants all SLOWER: tout aliased into dead V3
   (+2..17us: V3 rewrite waits the 6.6us store), stores on ACT queue
   (+50us: non-SP DMA path is slow), tin ring 5/6 (+15us: deeper
   prefetch bursts more DMA against the DVE), tout bufs=3 (no change).
 - Run-to-run noise of the repeat-slope measurement is +/-5us; best
   observed 87.6us, band 87-96us.
"""

import numpy as np

import concourse.bass as bass
import concourse.bacc as bacc
import concourse.mybir as mybir
import concourse.dve_ops as dve_ops
from concourse.dve_spec import Spec, Src0, Src1, C0, Zero, select, lower
from concourse.dve_uop import DveOpSpec
from concourse.tile import TileContext
from concourse.bass_utils import run_bass_kernel_spmd


def _register_nms_select():
    """Fused NMS select as a custom DVE op:
        out = Src0 if (Src1 - Src0) < s0 else 0      (Src0=p, Src1=V=3x3max)
    With s0 = 2^-24: V - p is exact in fp32 (inputs are multiples of 2^-23),
    zero iff p is the window max, else >= 2^-23 -> bit-exact select in ONE
    DVE pass, replacing sub + scalar_tensor_tensor + ACT relu."""
    name = "NMS_SELECT_ANT"
    if name in dve_ops._SUB_OPCODE_FOR_NAME:
        return next(o for o in dve_ops.OPS if o.name == name)
    spec = Spec(
        body=select(Src1 - Src0 < C0, Src0, Zero),
        reference=lambda in0, in1, s0, s1, imm2: np.where(
            (in1.astype(np.float32).reshape(in0.shape) - in0) < s0, in0, 0.0
        ).astype(np.float32),
    )
    # Self-pin the uops sha (the pin exists to catch lowering drift of
    # in-repo ops; for a runtime-registered op we pin to what we lower now).
    shas = {}
    for ver in ("v3", "v4"):
        try:
            s = DveOpSpec(name=name, opcode=0, uops=lower(spec, ver=ver),
                          rd1_en=True)
            shas[ver] = s.sha(ver)
        except Exception:
            pass
    op = dve_ops.DveOp(name, spec, subdim=False, uops_sha=shas)
    row = max(dve_ops._SUB_OPCODE_FOR_NAME.values()) + 1
    assert row < 0x20
    dve_ops.OPS.append(op)
    dve_ops.CUSTOM_DVE_SPECS[name] = spec
    dve_ops._SUB_OPCODE_FOR_NAME[name] = row
    return op


NMS_SELECT = _register_nms_select()
EPS_SEL = float(2.0 ** -24)

B, C, H, W = 16, 80, 128, 128
NCORES = 8
PLANES = B * C            # 1280
PPC = PLANES // NCORES    # 160 planes per core
GP = 32                   # planes per tile-group
NST = 4                   # vertical strips per plane
SR = H // NST             # 32 output rows per strip
NG = PPC // GP            # 5 groups per core
HP = H + 2                # 130 padded
WP = W + 2                # 130 padded
F32 = mybir.dt.float32

_CACHE = {}
LAST_RESULT = None        # BassKernelResults of the most recent run

TIN_P = (SR + 2) * WP   # tin partition stride (34*130)
V3_P = SR * WP          # V3 partition stride (32*130)
Q_P = (SR // 2) * WP    # Q partition stride (16*130)
TOUT_P = SR * W         # tout / V partition stride (32*128)
W2 = W // 2


def _ap(t, pstride, off, dims):
    """Strided view of a tile: dims = [[step, count], ...] appended after the
    128-partition dim."""
    return bass.AP(t.tensor, t.offset + off, [[pstride, 128]] + dims)


class _GroupTiles:
    """SBUF tiles for one 32-plane group plus the 7 full-size sweep emitters."""

    def __init__(self, nc, pool, tin, idx):
        self.nc = nc
        self.tin = tin
        self.Qv = pool.tile([128, SR // 2, WP], F32, tag=f"Qv{idx}", bufs=1,
                            name=f"Qv{idx}")
        self.V3 = pool.tile([128, SR, WP], F32, tag=f"V3{idx}", bufs=1,
                            name=f"V3{idx}")
        self.Ph = self.Qv  # aliased: Q is dead once V3 is built
        self.V = pool.tile([128, SR, W], F32, tag=f"V{idx}", bufs=1,
                           name=f"V{idx}")
        # (Aliasing tout into dead V3 space: 1.6-17us slower (store-DMA
        # WAR on the V3 rewrite); tout bufs=3: no change; deeper tin ring
        # (5/6): 15us slower (DMA burst/bank contention vs DVE streams).
        self.tout = pool.tile([128, SR, W], F32, tag="tout", bufs=2,
                              name="tout")

    # Each emitter takes a (k0, k1) pair-index range (vertical ops) or
    # (r0, r1) row range (horizontal ops); full-size = the whole range.
    # (Narrowing q/e/o to 128 cols with Pool-engine memsets for the static-
    # zero V3 edge cols was measured ~8µs SLOWER: GPSIMD shares the DVE SBUF
    # port and the cross-engine semaphores outweigh the 96-cycle saving.)
    def q(self, k0, k1):
        # Q[k] = max(tin[2k+1], tin[2k+2])
        n = k1 - k0
        self.nc.vector.tensor_max(
            _ap(self.Qv, Q_P, k0 * WP, [[WP, n], [1, WP]]),
            _ap(self.tin, TIN_P, (2 * k0 + 1) * WP, [[2 * WP, n], [1, WP]]),
            _ap(self.tin, TIN_P, (2 * k0 + 2) * WP, [[2 * WP, n], [1, WP]]),
        )

    def e(self, k0, k1):
        # V3[2k] = max(tin[2k], Q[k])
        n = k1 - k0
        self.nc.vector.tensor_max(
            _ap(self.V3, V3_P, (2 * k0) * WP, [[2 * WP, n], [1, WP]]),
            _ap(self.tin, TIN_P, (2 * k0) * WP, [[2 * WP, n], [1, WP]]),
            _ap(self.Qv, Q_P, k0 * WP, [[WP, n], [1, WP]]),
        )

    def o(self, k0, k1):
        # V3[2k+1] = max(Q[k], tin[2k+3])
        n = k1 - k0
        self.nc.vector.tensor_max(
            _ap(self.V3, V3_P, (2 * k0 + 1) * WP, [[2 * WP, n], [1, WP]]),
            _ap(self.Qv, Q_P, k0 * WP, [[WP, n], [1, WP]]),
            _ap(self.tin, TIN_P, (2 * k0 + 3) * WP, [[2 * WP, n], [1, WP]]),
        )

    def p(self, r0, r1):
        # Ph[m] = max(V3[:,2m+1], V3[:,2m+2]); Ph is a [SR, W2] view of Qv
        n = r1 - r0
        self.nc.vector.tensor_max(
            _ap(self.Ph, Q_P, r0 * W2, [[W2, n], [1, W2]]),
            _ap(self.V3, V3_P, r0 * WP + 1, [[WP, n], [2, W2]]),
            _ap(self.V3, V3_P, r0 * WP + 2, [[WP, n], [2, W2]]),
        )

    def ve(self, r0, r1):
        # V[2m] = max(V3[:,2m], Ph[m])
        n = r1 - r0
        self.nc.vector.tensor_max(
            _ap(self.V, TOUT_P, r0 * W, [[W, n], [2, W2]]),
            _ap(self.V3, V3_P, r0 * WP, [[WP, n], [2, W2]]),
            _ap(self.Ph, Q_P, r0 * W2, [[W2, n], [1, W2]]),
        )

    def vo(self, r0, r1):
        # V[2m+1] = max(Ph[m], V3[:,2m+3])
        n = r1 - r0
        self.nc.vector.tensor_max(
            _ap(self.V, TOUT_P, r0 * W + 1, [[W, n], [2, W2]]),
            _ap(self.Ph, Q_P, r0 * W2, [[W2, n], [1, W2]]),
            _ap(self.V3, V3_P, r0 * WP + 3, [[WP, n], [2, W2]]),
        )

    def s(self, r0, r1):
        # out = select(V - p < eps, p, 0)
        n = r1 - r0
        self.nc.vector._custom_dve(
            NMS_SELECT,
            out=_ap(self.tout, TOUT_P, r0 * W, [[W, n], [1, W]]),
            in0=_ap(self.tin, TIN_P, (r0 + 1) * WP + 1, [[WP, n], [1, W]]),
            in1=_ap(self.V, TOUT_P, r0 * W, [[W, n], [1, W]]),
            s0=EPS_SEL,
        )


def _emit_pair(a: _GroupTiles, b: _GroupTiles):
    """Two groups interleaved, full-size ops: every producer->consumer pair
    is >= 2 instructions apart.  14 instructions / 2 groups."""
    K, R = SR // 2, SR
    a.q(0, K); b.q(0, K)
    a.e(0, K); a.o(0, K)
    b.e(0, K); b.o(0, K)
    a.p(0, R); b.p(0, R)
    a.ve(0, R); b.ve(0, R)
    a.vo(0, R); b.vo(0, R)
    a.s(0, R); b.s(0, R)


def _emit_single(a: _GroupTiles):
    """Odd trailing group: staggered halves, every dep >= 2 apart."""
    KK = [(0, SR // 4), (SR // 4, SR // 2)]
    HH = [(0, SR // 2), (SR // 2, SR)]
    a.q(*KK[0]); a.q(*KK[1])
    a.e(*KK[0]); a.o(*KK[0])
    a.e(*KK[1]); a.o(*KK[1])
    a.p(*HH[0]); a.p(*HH[1])
    a.ve(*HH[0]); a.ve(*HH[1])
    a.vo(*HH[0]); a.vo(*HH[1])
    a.s(*HH[0]); a.s(*HH[1])


def _build_program(repeat: int = 1, mode: str = "full"):
    # Bacc (not raw Bass): its compile pipeline runs generate_event_semaphores,
    # which splits multi-wait instructions to satisfy the TRN2 1-wait-per-
    # instruction ISA constraint.
    nc = bacc.Bacc()
    x = nc.dram_tensor("x", [PPC, HP, WP], F32, kind="ExternalInput")
    y = nc.dram_tensor("y", [PPC, H, W], F32, kind="ExternalOutput")
    xap = x[:]
    yap = y[:]

    glist = [g for _ in range(repeat) for g in range(NG)]
    tins = {}
    NLOAD = 2 if mode == "contend" else 4  # tin ring: 2 in compute + 2 prefetching

    def _emit_load(gi):
        # DRAM side iterates (plane, strip, row, col) so that partition
        # p = plane*NST + strip; strips overlap by 2 rows.  Plane (count 32)
        # outermost: the HWDGE queue fan-out keys on the outer dim, and 32
        # spreads across all rings (3x DMA BW vs strip-outermost).
        t = pool.tile([128, SR + 2, WP], F32, tag="tin", bufs=NLOAD, name="tin")
        src = bass.AP(
            xap.tensor,
            glist[gi] * GP * HP * WP,
            [[HP * WP, GP], [SR * WP, NST], [1, (SR + 2) * WP]],
        )
        if mode == "nodma":
            nc.gpsimd.memset(t[:], 0.0)
        else:
            nc.sync.dma_start(out=t[:], in_=src)
        tins[gi] = t

    def _store(g, t):
        dst = bass.AP(
            yap.tensor,
            g * GP * H * W,
            [[H * W, GP], [SR * W, NST], [1, SR * W]],
        )
        # (Issuing stores from the ACT engine's DMA queue instead was
        # measured 143us vs 93.7 -- the non-SP queues go through a slow
        # path; keep every DMA on nc.sync.)
        nc.sync.dma_start(out=dst, in_=t[:])

    with TileContext(nc) as tc:
        with tc.tile_pool(name="pool", bufs=1) as pool:
            n = len(glist)
            if mode in ("pure", "contend"):
                # Compute-only diagnostic: load a fixed ring once, then run
                # every group's sweeps against those resident tiles (no DMA
                # data-dependencies with compute).  "contend" additionally
                # issues the full load/store DMA traffic against dummy tiles
                # so SBUF port contention is present but sync stalls are not.
                ring = []
                for j in range(NLOAD):
                    _emit_load(j)
                    ring.append(tins[j])
                tins.clear()
                for gi in range(n):
                    tins[gi] = ring[gi % NLOAD]
                if mode == "contend":
                    do = pool.tile([128, SR, W], F32, tag="dout",
                                   bufs=1, name="dout")
                    nc.vector.memset(do[:], 0.0)
                    douts = [do, do]  # stores only read: no hazards
                    for gi in range(n):
                        d = pool.tile([128, SR + 2, WP], F32, tag="dummy",
                                      bufs=2, name="dummy")
                        src = bass.AP(
                            xap.tensor,
                            glist[gi] * GP * HP * WP,
                            [[HP * WP, GP], [SR * WP, NST],
                             [1, (SR + 2) * WP]],
                        )
                        nc.sync.dma_start(out=d[:], in_=src)
                        dst = bass.AP(
                            yap.tensor,
                            glist[gi] * GP * H * W,
                            [[H * W, GP], [SR * W, NST], [1, SR * W]],
                        )
                        nc.sync.dma_start(out=dst, in_=douts[gi % 2][:])
            else:
                for j in range(min(NLOAD, n)):
                    _emit_load(j)
            i = 0
            while i < n:
                pair = i + 1 < n
                # Next loads before this block's stores: the in-order SP
                # queue must never hold a needed load behind a store's wait.
                for j in range(i + 2, min(i + (4 if pair else 3), n)):
                    if mode not in ("pure", "contend") and (
                        j >= NLOAD or j not in tins
                    ):
                        _emit_load(j)
                if pair:
                    ga = _GroupTiles(nc, pool, tins.pop(i), 0)
                    gb = _GroupTiles(nc, pool, tins.pop(i + 1), 1)
                    if mode == "dmaonly":
                        for off, gt in ((0, ga), (1, gb)):
                            tin_flat = _ap(gt.tin, TIN_P, 0, [[1, SR * W]])
                            dst = bass.AP(
                                yap.tensor,
                                glist[i + off] * GP * H * W,
                                [[H * W, GP], [SR * W, NST], [1, SR * W]],
                            )
                            nc.sync.dma_start(out=dst, in_=tin_flat)
                        i += 2
                        continue
                    _emit_pair(ga, gb)
                    if mode != "pure":
                        _store(glist[i], ga.tout)
                        _store(glist[i + 1], gb.tout)
                    i += 2
                else:
                    ga = _GroupTiles(nc, pool, tins.pop(i), 0)
                    if mode == "dmaonly":
                        tin_flat = _ap(ga.tin, TIN_P, 0, [[1, SR * W]])
                        dst = bass.AP(
                            yap.tensor,
                            glist[i] * GP * H * W,
                            [[H * W, GP], [SR * W, NST], [1, SR * W]],
                        )
                        nc.sync.dma_start(out=dst, in_=tin_flat)
                        i += 1
                        continue
                    _emit_single(ga)
                    if mode != "pure":
                        _store(glist[i], ga.tout)
                    i += 1
    nc.finalize()
    return nc


def get_nc(repeat: int = 1, mode: str = "full"):
    key = f"nc{repeat}_{mode}"
    if key not in _CACHE:
        _CACHE[key] = _build_program(repeat, mode)
    return _CACHE[key]


def pad_input(points: np.ndarray) -> np.ndarray:
    pts = np.ascontiguousarray(points, dtype=np.float32).reshape(PLANES, H, W)
    xpad = np.zeros((PLANES, HP, WP), np.float32)
    xpad[:, 1:H + 1, 1:W + 1] = pts
    return xpad


def kernel(**inputs) -> np.ndarray:
    global LAST_RESULT
    import os

    # The axon NTFF profile hook is absent in this environment; force the
    # non-tracing execute path even if BASS_TRACE is set externally.
    os.environ["BASS_NEVER_TRACE"] = "1"
    xpad = pad_input(inputs["points"])
    nc = get_nc()
    in_maps = [{"x": xpad[k * PPC:(k + 1) * PPC]} for k in range(NCORES)]
    res = run_bass_kernel_spmd(nc, in_maps, list(range(NCORES)))
    LAST_RESULT = res
    full = np.empty((PLANES, H, W), np.float32)
    for k in range(NCORES):
        full[k * PPC:(k + 1) * PPC] = res.results[k]["y"]
    return full.reshape(B, C, H, W)



# revision 34
# speedup vs baseline: 1.3435x; 1.3435x over previous
"""3x3 zero-padded window NMS (CenterNet points) on 8 trn2 NeuronCores.

points: [16, 80, 128, 128] f32 in [0,1).  out = where(p == 3x3_local_max, p, 0).

Strategy
--------
Pure data parallel over the 1280 (b,c) planes: core k owns planes
[160k, 160k+160).  Host zero-pads each plane to 130x130 so the kernel has
no edge cases.

Per-core layout: planes on SBUF partitions.  A tile covers 32 planes x
4 vertical strips (= 128 partitions), each strip 32 output rows + 2 halo
rows, full 130-col width.  All shifts are free-dim AP shifts.

Compute (per tile, all exact fp32, all on DVE).  The DVE is the only
engine with 2-tensor elementwise ops (ACT bias/scale are per-partition
scalars; GPSIMD TensorTensor is rejected by walrus codegen on Pool), and
fp32 tensor_tensor runs at 1 elem/cycle/lane, so the cycle count is the
total number of output elements across sweeps.  A pair-max decomposition
of the 3-tap sliding max cuts that from 2/elem to 1.5/elem per direction:

  vertical   Q[k]     = max(t[2k+1], t[2k+2])            k = 0..15
             V3[2k]   = max(t[2k],   Q[k])       (rows 2k..2k+2)
             V3[2k+1] = max(Q[k],    t[2k+3])    (rows 2k+1..2k+3)
  horizontal Ph[m]    = max(V3[:,2m+1], V3[:,2m+2])      m = 0..63
             V[2m]    = max(V3[:,2m], Ph[m])     (cols 2m..2m+2)
             V[2m+1]  = max(Ph[m], V3[:,2m+3])   (cols 2m+1..2m+3)
  out = select(V - p < 2^-24, p, 0)              fused custom DVE op

16480 cycles/group vs 20770 for the plain separable 2+2-pass form.  Every
sweep writes a tile it does not read: an in-place V3 update was measured
~16% SLOWER end-to-end (read+write streams on the same SBUF bank).  Ph is
aliased into Qv's storage (2080 elems/partition >= 2048; Q is dead once
V3 is built) to keep the footprint under the SBUF budget.  (A chained-
select variant -- out = SEL(V3_interleaved, SEL(Ph_expanded, p)) -- would
drop one instruction at equal FD, but _custom_dve APs are capped at 2 free
dims and the expanded/interleaved reads need 3.)
Inputs are multiples of 2^-23 (jax.random.uniform), so V - p is exact in
fp32: 0 iff p is the window max, else >= 2^-23 -> the select is bit-exact.

Perf notes (HW-measured):
 - The DVE stalls ~op-duration when an op consumes the *immediately*
   previous op's output; distance >= 2 streams at full rate.  Two groups
   are processed interleaved (Qg Qh Eg Og Eh Oh Pg Ph Veg Veh Vog Voh Sg
   Sh) so every op is full-size, 7 instructions/group, and every
   producer->consumer pair is >= 2 instructions apart -- halving the
   ~151-cycle-per-instruction init overhead vs split-half staggering.
   An odd trailing group falls back to the 14-half-instruction order.
 - DMA APs keep the 32-plane dim outermost (HWDGE ring fan-out keys on it;
   3x bandwidth vs strip-outermost).
 - Loads prefetch 2 groups ahead and are emitted before stores so the
   in-order SP queue never holds a needed load behind a store's wait.

Why this is the roofline (HW-bisected; all numbers per-iteration slope):
 - mode=pure (compute only, no DMA): 83.4us -> real DVE ~1.0 GHz, 82400
   cycles/core of work + ~1us init.  The DVE has 2x4B SBUF read ports;
   every fp32 2-src op is port-bound at 1 out/cycle.  The sweep structure
   reads 8.05 B/port-pair per output elem = the 2-src-op lower bound
   (vertical 1.5 + horizontal 1.5 + select 1.0 passes/elem).
 - Faster modes exist only for 1-src ops (InstTensorScalarPtr/TensorCopy
   support 2x_2p fp32 = 2 elem/cycle) -- nothing in NMS is 1-src.
   TensorTensor supports only 2x_1p (needs 2-byte dtype); 16-bit maxes
   break exactness (measured-style est. rel err ~0.2 vs 2e-2 gate: a
   false peak needs ~18 mantissa bits to stay under tolerance).
   InstTensorReduce/InstPool read 1 elem/cycle (no modes) -> worse.
   Custom DVE ops report no perf modes regardless of shape/subdim.
 - GPSIMD: walrus rejects ALL 2-src ops on Pool (TensorTensor AND
   scalar_tensor_tensor); only tensor_scalar/copy pass -> no offload.
   ACT: bias/scale per-partition scalars only -> no offload.
 - mode=dmaonly: 69.2us (21.8 MB @ ~315 GB/s) -> DMA fully hidden.
 - full - pure = 4-12us: DMA<->DVE SBUF bank contention, sensitive to
   burst pattern.  Variants all SLOWER: tout aliased into dead V3
   (+2..17us: V3 rewrite waits the 6.6us store), stores on ACT queue
   (+50us: non-SP DMA path is slow), tin ring 5/6 (+15us: deeper
   prefetch bursts more DMA against the DVE), tout bufs=3 (no change).
 - Run-to-run noise of the repeat-slope measurement is +/-5us; best
   observed 87.6us, band 87-96us.
"""

import numpy as np

import concourse.bass as bass
import concourse.bacc as bacc
import concourse.mybir as mybir
import concourse.dve_ops as dve_ops
from concourse.dve_spec import Spec, Src0, Src1, C0, Zero, select, lower
from concourse.dve_uop import (
    AluInp,
    AluOp,
    DelayInp,
    DveOpSpec,
    InpSel,
    OutPath,
    OutSel,
    Trigger,
    UopConfig,
    UopDpConfig,
)
from concourse.tile import TileContext
from concourse.bass_utils import run_bass_kernel_spmd


def _register_nms_select():
    """Fused NMS select as a custom DVE op:
        out = Src0 if (Src1 - Src0) < s0 else 0      (Src0=p, Src1=V=3x3max)
    With s0 = 2^-24: V - p is exact in fp32 (inputs are multiples of 2^-23),
    zero iff p is the window max, else >= 2^-23 -> bit-exact select in ONE
    DVE pass, replacing sub + scalar_tensor_tensor + ACT relu."""
    name = "NMS_SELECT_ANT"
    if name in dve_ops._SUB_OPCODE_FOR_NAME:
        return next(o for o in dve_ops.OPS if o.name == name)
    spec = Spec(
        body=select(Src1 - Src0 < C0, Src0, Zero),
        reference=lambda in0, in1, s0, s1, imm2: np.where(
            (in1.astype(np.float32).reshape(in0.shape) - in0) < s0, in0, 0.0
        ).astype(np.float32),
    )
    # Self-pin the uops sha (the pin exists to catch lowering drift of
    # in-repo ops; for a runtime-registered op we pin to what we lower now).
    shas = {}
    for ver in ("v3", "v4"):
        try:
            s = DveOpSpec(name=name, opcode=0, uops=lower(spec, ver=ver),
                          rd1_en=True)
            shas[ver] = s.sha(ver)
        except Exception:
            pass
    op = dve_ops.DveOp(name, spec, subdim=False, uops_sha=shas)
    row = max(dve_ops._SUB_OPCODE_FOR_NAME.values()) + 1
    assert row < 0x20
    dve_ops.OPS.append(op)
    dve_ops.CUSTOM_DVE_SPECS[name] = spec
    dve_ops._SUB_OPCODE_FOR_NAME[name] = row
    return op


NMS_SELECT = _register_nms_select()
EPS_SEL = float(2.0 ** -24)


def _build_tail_uops():
    """Single-uOp streaming program for the fused horizontal-max+select.

    Streams (per element i, two equal-length 130-elems/row streams):
      src0 = v   = V3 row r            (lane 1)
      src1 = pp  = tin row r+1         (lane 2)   [p with pad cols]
      lane 3 = CONST_0 (eps), lane 4 = ZERO.

    Cross-element taps via delay lanes loaded from CURR_ALU_OUT (a stage's
    own flop = its previous element's result; same mechanism as scan()):
      blk0  BYPASS v[i]          chain0@0 <- CURR = v[i-1]
      blk1  MAX(v[i], v[i-1])    chain0@1 <- CURR = p2[i-1]; chain4 <- v[i]
      blk2  BYPASS pp[i]         chain1@2 <- CURR = pp[i-1]
      blk3  MAX(v[i], p2[i-1])   = max(v[i-2..i])
      blk4  SUB(max3, pp[i-1])   pp[i-1] = p at out col i-2
      blk5  IS_LT(diff, eps)
      blk6  SELECT(zero, p)      pred = blk5 (bit0 of int bool)
      blk7  BYPASS -> WR0_LO

    Out position i = (r, c): valid for c >= 2 (x = c-2); c in {0,1} holds
    junk (stale cross-row/uninitialized taps) and is skipped by the
    compacting copy.  Splitting the stream at any row boundary is safe:
    the first 2 elements of a sub-stream land in junk cols.
    """
    u = UopConfig()
    u.enable_input(InpSel.SRC_0, 1)
    u.enable_input(InpSel.SRC_1, 2)
    u.enable_input(InpSel.CONST_0, 3)
    u.enable_input(InpSel.ZERO, 4)
    u.require_inp0 = 1
    u.require_inp1 = 1
    u.trigger = (Trigger.SRC_TENSOR_DONE, Trigger.NONE, Trigger.NONE)
    u.next_uop = (0, 0, 0)
    u.enable_output(OutSel.ALU_OUT, OutPath.WR0_LO)

    b = u.datapath_config
    # blk0: f0 = v[i]; tap chain0 = v[i-1]; pass pp/eps/zero lanes.
    b[0].enable_alu(AluOp.BYPASS, AluInp.PREV_DELAY_0)
    b[0].enable_delay_from_src(DelayInp.CURR_ALU_OUT, 0)
    b[0].pass_through_delay(1, 2, 3)
    # blk1: f1 = p2[i] = max(v[i], v[i-1]); taps: chain0 = p2[i-1],
    # chain4 = v[i] (from blk0's flop).
    b[1].enable_alu(AluOp.MAX, AluInp.PREV_ALU_OUT, AluInp.PREV_DELAY_0)
    b[1].enable_delay_from_src(DelayInp.CURR_ALU_OUT, 0)
    b[1].enable_delay_from_src(DelayInp.PREV_ALU_OUT, 4)
    b[1].pass_through_delay(1, 2, 3)
    # blk2: f2 = pp[i]; tap chain1 = pp[i-1]; pass the rest.
    b[2].enable_alu(AluOp.BYPASS, AluInp.PREV_DELAY_1)
    b[2].enable_delay_from_src(DelayInp.CURR_ALU_OUT, 1)
    b[2].pass_through_delay(0, 2, 3, 4)
    # blk3: f3 = max3 = max(v[i], p2[i-1]).
    b[3].enable_alu(AluOp.MAX, AluInp.PREV_DELAY_4, AluInp.PREV_DELAY_0)
    b[3].pass_through_delay(1, 2, 3)
    # blk4: f4 = max3 - p.
    b[4].enable_alu(AluOp.SUBTRACT, AluInp.PREV_ALU_OUT, AluInp.PREV_DELAY_1)
    b[4].pass_through_delay(1, 2, 3)
    # blk5: f5 = (diff < eps)  [integer bool: bit0 valid for SELECT]
    b[5].enable_alu(AluOp.IS_LT, AluInp.PREV_ALU_OUT, AluInp.PREV_DELAY_2)
    b[5].pass_through_delay(1, 3)
    # blk6: f6 = pred ? p : 0   (SELECT: true -> alu_src1)
    b[6].enable_alu(AluOp.SELECT, AluInp.PREV_DELAY_3, AluInp.PREV_DELAY_1)
    # blk7: carry to the write stage.
    b[7].enable_alu(AluOp.BYPASS, AluInp.PREV_ALU_OUT)
    return [u]


def _register_nms_tail():
    """out[r, c] = select(max(V3[r, c-2..c]) - tin[r+1, c-1] < s0,
                          tin[r+1, c-1], 0) for c >= 2; junk for c < 2.
    Hand-written uOps (the Spec DSL has no sliding-window delay taps);
    the DSL body below is a never-lowered placeholder -- _COMPILE_CACHE
    is prefilled so DveOp.compile() returns the hand program."""
    name = "NMS_TAIL_ANT"
    if name in dve_ops._SUB_OPCODE_FOR_NAME:
        return next(o for o in dve_ops.OPS if o.name == name)

    def _ref(in0, in1, s0, s1, imm2):
        v = np.asarray(in0, np.float32).reshape(in0.shape[0], -1, WP)
        pp = np.asarray(in1, np.float32).reshape(v.shape)
        out = np.zeros_like(v)
        m3 = np.maximum(np.maximum(v[:, :, 2:], v[:, :, 1:-1]), v[:, :, :-2])
        p = pp[:, :, 1:-1]
        s0 = float(s0) if not isinstance(s0, np.ndarray) else s0.reshape(-1, 1, 1)
        out[:, :, 2:] = np.where((m3 - p) < s0, p, 0.0)
        return out

    spec = Spec(body=select(Src1 - Src0 < C0, Src0, Zero), reference=_ref)
    row = max(dve_ops._SUB_OPCODE_FOR_NAME.values()) + 1
    assert row < 0x20
    uops = _build_tail_uops()
    shas = {}
    for ver in ("v3", "v4"):
        try:
            s = DveOpSpec(name=name, opcode=row, uops=uops, rd1_en=True)
            shas[ver] = s.sha(ver)
            dve_ops._COMPILE_CACHE[(name, ver)] = s
        except Exception:
            pass
    assert "v3" in shas, "tail uops failed v3 packing"
    op = dve_ops.DveOp(name, spec, subdim=False, uops_sha=shas)
    dve_ops.OPS.append(op)
    dve_ops.CUSTOM_DVE_SPECS[name] = spec
    dve_ops._SUB_OPCODE_FOR_NAME[name] = row
    return op


NMS_TAIL = _register_nms_tail()

B, C, H, W = 16, 80, 128, 128
NCORES = 8
PLANES = B * C            # 1280
PPC = PLANES // NCORES    # 160 planes per core
GP = 32                   # planes per tile-group
NST = 4                   # vertical strips per plane
SR = H // NST             # 32 output rows per strip
NG = PPC // GP            # 5 groups per core
HP = H + 2                # 130 padded
WP = W + 2                # 130 padded
F32 = mybir.dt.float32

_CACHE = {}
LAST_RESULT = None        # BassKernelResults of the most recent run

USE_TAIL = True         # fused streaming tail op (f+c) vs p/ve/vo/s

TIN_P = (SR + 2) * WP   # tin partition stride (34*130)
V3_P = SR * WP          # V3 partition stride (32*130)
Q_P = (SR // 2) * WP    # Q partition stride (16*130)
TOUT_P = SR * W         # tout / V partition stride (32*128)
W2 = W // 2


def _ap(t, pstride, off, dims):
    """Strided view of a tile: dims = [[step, count], ...] appended after the
    128-partition dim."""
    return bass.AP(t.tensor, t.offset + off, [[pstride, 128]] + dims)


class _GroupTiles:
    """SBUF tiles for one 32-plane group plus the 7 full-size sweep emitters."""

    def __init__(self, nc, pool, tin, idx):
        self.nc = nc
        self.tin = tin
        self.Qv = pool.tile([128, SR // 2, WP], F32, tag=f"Qv{idx}", bufs=1,
                            name=f"Qv{idx}")
        self.V3 = pool.tile([128, SR, WP], F32, tag=f"V3{idx}", bufs=1,
                            name=f"V3{idx}")
        self.Ph = self.Qv  # aliased: Q is dead once V3 is built
        if USE_TAIL:
            # t2: 130-wide fused-tail output (cols 0-1 junk per row)
            self.t2 = pool.tile([128, SR, WP], F32, tag=f"T2{idx}", bufs=1,
                                name=f"T2{idx}")
        else:
            self.V = pool.tile([128, SR, W], F32, tag=f"V{idx}", bufs=1,
                               name=f"V{idx}")
        # (Aliasing tout into dead V3 space: 1.6-17us slower (store-DMA
        # WAR on the V3 rewrite); tout bufs=3: no change; deeper tin ring
        # (5/6): 15us slower (DMA burst/bank contention vs DVE streams).
        self.tout = pool.tile([128, SR, W], F32, tag="tout", bufs=2,
                              name="tout")

    # Each emitter takes a (k0, k1) pair-index range (vertical ops) or
    # (r0, r1) row range (horizontal ops); full-size = the whole range.
    # (Narrowing q/e/o to 128 cols with Pool-engine memsets for the static-
    # zero V3 edge cols was measured ~8µs SLOWER: GPSIMD shares the DVE SBUF
    # port and the cross-engine semaphores outweigh the 96-cycle saving.)
    def q(self, k0, k1):
        # Q[k] = max(tin[2k+1], tin[2k+2])
        n = k1 - k0
        self.nc.vector.tensor_max(
            _ap(self.Qv, Q_P, k0 * WP, [[WP, n], [1, WP]]),
            _ap(self.tin, TIN_P, (2 * k0 + 1) * WP, [[2 * WP, n], [1, WP]]),
            _ap(self.tin, TIN_P, (2 * k0 + 2) * WP, [[2 * WP, n], [1, WP]]),
        )

    def e(self, k0, k1):
        # V3[2k] = max(tin[2k], Q[k])
        n = k1 - k0
        self.nc.vector.tensor_max(
            _ap(self.V3, V3_P, (2 * k0) * WP, [[2 * WP, n], [1, WP]]),
            _ap(self.tin, TIN_P, (2 * k0) * WP, [[2 * WP, n], [1, WP]]),
            _ap(self.Qv, Q_P, k0 * WP, [[WP, n], [1, WP]]),
        )

    def o(self, k0, k1):
        # V3[2k+1] = max(Q[k], tin[2k+3])
        n = k1 - k0
        self.nc.vector.tensor_max(
            _ap(self.V3, V3_P, (2 * k0 + 1) * WP, [[2 * WP, n], [1, WP]]),
            _ap(self.Qv, Q_P, k0 * WP, [[WP, n], [1, WP]]),
            _ap(self.tin, TIN_P, (2 * k0 + 3) * WP, [[2 * WP, n], [1, WP]]),
        )

    def p(self, r0, r1):
        # Ph[m] = max(V3[:,2m+1], V3[:,2m+2]); Ph is a [SR, W2] view of Qv
        n = r1 - r0
        self.nc.vector.tensor_max(
            _ap(self.Ph, Q_P, r0 * W2, [[W2, n], [1, W2]]),
            _ap(self.V3, V3_P, r0 * WP + 1, [[WP, n], [2, W2]]),
            _ap(self.V3, V3_P, r0 * WP + 2, [[WP, n], [2, W2]]),
        )

    def ve(self, r0, r1):
        # V[2m] = max(V3[:,2m], Ph[m])
        n = r1 - r0
        self.nc.vector.tensor_max(
            _ap(self.V, TOUT_P, r0 * W, [[W, n], [2, W2]]),
            _ap(self.V3, V3_P, r0 * WP, [[WP, n], [2, W2]]),
            _ap(self.Ph, Q_P, r0 * W2, [[W2, n], [1, W2]]),
        )

    def vo(self, r0, r1):
        # V[2m+1] = max(Ph[m], V3[:,2m+3])
        n = r1 - r0
        self.nc.vector.tensor_max(
            _ap(self.V, TOUT_P, r0 * W + 1, [[W, n], [2, W2]]),
            _ap(self.Ph, Q_P, r0 * W2, [[W2, n], [1, W2]]),
            _ap(self.V3, V3_P, r0 * WP + 3, [[WP, n], [2, W2]]),
        )

    def s(self, r0, r1):
        # out = select(V - p < eps, p, 0)
        n = r1 - r0
        self.nc.vector._custom_dve(
            NMS_SELECT,
            out=_ap(self.tout, TOUT_P, r0 * W, [[W, n], [1, W]]),
            in0=_ap(self.tin, TIN_P, (r0 + 1) * WP + 1, [[WP, n], [1, W]]),
            in1=_ap(self.V, TOUT_P, r0 * W, [[W, n], [1, W]]),
            s0=EPS_SEL,
        )

    def f(self, r0, r1):
        # t2[r, c] = select(max(V3[r,c-2..c]) - p[r,c-2] < eps, p, 0),
        # one streaming pass; cols 0,1 of each row are junk.
        n = r1 - r0
        self.nc.vector._custom_dve(
            NMS_TAIL,
            out=_ap(self.t2, V3_P, r0 * WP, [[WP, n], [1, WP]]),
            in0=_ap(self.V3, V3_P, r0 * WP, [[WP, n], [1, WP]]),
            in1=_ap(self.tin, TIN_P, (r0 + 1) * WP, [[WP, n], [1, WP]]),
            s0=EPS_SEL,
        )

    def c(self, r0, r1):
        # Compact t2's valid cols 2..129 into the contiguous store tile
        # (InstTensorCopy: 2x_2p -> 2 elem/cycle fp32).
        n = r1 - r0
        self.nc.vector.tensor_copy(
            _ap(self.tout, TOUT_P, r0 * W, [[W, n], [1, W]]),
            _ap(self.t2, V3_P, r0 * WP + 2, [[WP, n], [1, W]]),
        )


def _emit_pair(a: _GroupTiles, b: _GroupTiles):
    """Two groups interleaved, full-size ops: every producer->consumer pair
    is >= 2 instructions apart.  10 (fused tail) or 14 instructions / 2
    groups."""
    K, R = SR // 2, SR
    a.q(0, K); b.q(0, K)
    a.e(0, K); a.o(0, K)
    b.e(0, K); b.o(0, K)
    if USE_TAIL:
        a.f(0, R); b.f(0, R)
        a.c(0, R); b.c(0, R)
        return
    a.p(0, R); b.p(0, R)
    a.ve(0, R); b.ve(0, R)
    a.vo(0, R); b.vo(0, R)
    a.s(0, R); b.s(0, R)


def _emit_single(a: _GroupTiles):
    """Odd trailing group: staggered halves, every dep >= 2 apart."""
    KK = [(0, SR // 4), (SR // 4, SR // 2)]
    HH = [(0, SR // 2), (SR // 2, SR)]
    a.q(*KK[0]); a.q(*KK[1])
    a.e(*KK[0]); a.o(*KK[0])
    a.e(*KK[1]); a.o(*KK[1])
    if USE_TAIL:
        # Sub-stream starts at a row boundary: the 2 stale-tap elements
        # land in that row's junk cols -- split is safe.
        a.f(*HH[0]); a.f(*HH[1])
        a.c(*HH[0]); a.c(*HH[1])
        return
    a.p(*HH[0]); a.p(*HH[1])
    a.ve(*HH[0]); a.ve(*HH[1])
    a.vo(*HH[0]); a.vo(*HH[1])
    a.s(*HH[0]); a.s(*HH[1])


def _build_program(repeat: int = 1, mode: str = "full"):
    # Bacc (not raw Bass): its compile pipeline runs generate_event_semaphores,
    # which splits multi-wait instructions to satisfy the TRN2 1-wait-per-
    # instruction ISA constraint.
    nc = bacc.Bacc()
    x = nc.dram_tensor("x", [PPC, HP, WP], F32, kind="ExternalInput")
    y = nc.dram_tensor("y", [PPC, H, W], F32, kind="ExternalOutput")
    xap = x[:]
    yap = y[:]

    glist = [g for _ in range(repeat) for g in range(NG)]
    tins = {}
    NLOAD = 2 if mode == "contend" else 4  # tin ring: 2 in compute + 2 prefetching

    def _emit_load(gi):
        # DRAM side iterates (plane, strip, row, col) so that partition
        # p = plane*NST + strip; strips overlap by 2 rows.  Plane (count 32)
        # outermost: the HWDGE queue fan-out keys on the outer dim, and 32
        # spreads across all rings (3x DMA BW vs strip-outermost).
        t = pool.tile([128, SR + 2, WP], F32, tag="tin", bufs=NLOAD, name="tin")
        src = bass.AP(
            xap.tensor,
            glist[gi] * GP * HP * WP,
            [[HP * WP, GP], [SR * WP, NST], [1, (SR + 2) * WP]],
        )
        if mode == "nodma":
            nc.gpsimd.memset(t[:], 0.0)
        else:
            nc.sync.dma_start(out=t[:], in_=src)
        tins[gi] = t

    def _store(g, t):
        dst = bass.AP(
            yap.tensor,
            g * GP * H * W,
            [[H * W, GP], [SR * W, NST], [1, SR * W]],
        )
        # (Issuing stores from the ACT engine's DMA queue instead was
        # measured 143us vs 93.7 -- the non-SP queues go through a slow
        # path; keep every DMA on nc.sync.)
        nc.sync.dma_start(out=dst, in_=t[:])

    with TileContext(nc) as tc:
        with tc.tile_pool(name="pool", bufs=1) as pool:
            n = len(glist)
            if mode in ("pure", "contend"):
                # Compute-only diagnostic: load a fixed ring once, then run
                # every group's sweeps against those resident tiles (no DMA
                # data-dependencies with compute).  "contend" additionally
                # issues the full load/store DMA traffic against dummy tiles
                # so SBUF port contention is present but sync stalls are not.
                ring = []
                for j in range(NLOAD):
                    _emit_load(j)
                    ring.append(tins[j])
                tins.clear()
                for gi in range(n):
                    tins[gi] = ring[gi % NLOAD]
                if mode == "contend":
                    do = pool.tile([128, SR, W], F32, tag="dout",
                                   bufs=1, name="dout")
                    nc.vector.memset(do[:], 0.0)
                    douts = [do, do]  # stores only read: no hazards
                    for gi in range(n):
                        d = pool.tile([128, SR + 2, WP], F32, tag="dummy",
                                      bufs=2, name="dummy")
                        src = bass.AP(
                            xap.tensor,
                            glist[gi] * GP * HP * WP,
                            [[HP * WP, GP], [SR * WP, NST],
                             [1, (SR + 2) * WP]],
                        )
                        nc.sync.dma_start(out=d[:], in_=src)
                        dst = bass.AP(
                            yap.tensor,
                            glist[gi] * GP * H * W,
                            [[H * W, GP], [SR * W, NST], [1, SR * W]],
                        )
                        nc.sync.dma_start(out=dst, in_=douts[gi % 2][:])
            else:
                for j in range(min(NLOAD, n)):
                    _emit_load(j)
            i = 0
            while i < n:
                pair = i + 1 < n
                # Next loads before this block's stores: the in-order SP
                # queue must never hold a needed load behind a store's wait.
                for j in range(i + 2, min(i + (4 if pair else 3), n)):
                    if mode not in ("pure", "contend") and (
                        j >= NLOAD or j not in tins
                    ):
                        _emit_load(j)
                if pair:
                    ga = _GroupTiles(nc, pool, tins.pop(i), 0)
                    gb = _GroupTiles(nc, pool, tins.pop(i + 1), 1)
                    if mode == "dmaonly":
                        for off, gt in ((0, ga), (1, gb)):
                            tin_flat = _ap(gt.tin, TIN_P, 0, [[1, SR * W]])
                            dst = bass.AP(
                                yap.tensor,
                                glist[i + off] * GP * H * W,
                                [[H * W, GP], [SR * W, NST], [1, SR * W]],
                            )
                            nc.sync.dma_start(out=dst, in_=tin_flat)
                        i += 2
                        continue
                    _emit_pair(ga, gb)
                    if mode != "pure":
                        _store(glist[i], ga.tout)
                        _store(glist[i + 1], gb.tout)
                    i += 2
                else:
                    ga = _GroupTiles(nc, pool, tins.pop(i), 0)
                    if mode == "dmaonly":
                        tin_flat = _ap(ga.tin, TIN_P, 0, [[1, SR * W]])
                        dst = bass.AP(
                            yap.tensor,
                            glist[i] * GP * H * W,
                            [[H * W, GP], [SR * W, NST], [1, SR * W]],
                        )
                        nc.sync.dma_start(out=dst, in_=tin_flat)
                        i += 1
                        continue
                    _emit_single(ga)
                    if mode != "pure":
                        _store(glist[i], ga.tout)
                    i += 1
    nc.finalize()
    return nc


def get_nc(repeat: int = 1, mode: str = "full"):
    key = f"nc{repeat}_{mode}"
    if key not in _CACHE:
        _CACHE[key] = _build_program(repeat, mode)
    return _CACHE[key]


def pad_input(points: np.ndarray) -> np.ndarray:
    pts = np.ascontiguousarray(points, dtype=np.float32).reshape(PLANES, H, W)
    xpad = np.zeros((PLANES, HP, WP), np.float32)
    xpad[:, 1:H + 1, 1:W + 1] = pts
    return xpad


def kernel(**inputs) -> np.ndarray:
    global LAST_RESULT
    import os

    # The axon NTFF profile hook is absent in this environment; force the
    # non-tracing execute path even if BASS_TRACE is set externally.
    os.environ["BASS_NEVER_TRACE"] = "1"
    xpad = pad_input(inputs["points"])
    nc = get_nc()
    in_maps = [{"x": xpad[k * PPC:(k + 1) * PPC]} for k in range(NCORES)]
    res = run_bass_kernel_spmd(nc, in_maps, list(range(NCORES)))
    LAST_RESULT = res
    full = np.empty((PLANES, H, W), np.float32)
    for k in range(NCORES):
        full[k * PPC:(k + 1) * PPC] = res.results[k]["y"]
    return full.reshape(B, C, H, W)

